# revision 3
# baseline (speedup 1.0000x reference)
"""2-layer GCN on 8 trn2 cores — dma_gather (ucode) + one-hot matmul reduce.

Design (single SPMD program, ~3.3 ms vs 6.77 ms indirect-DMA baseline):
  - Nodes ranked by degree, dealt round-robin to cores. Table row of node n:
    row = owner*SH + pos (SH=12800, TAB=102400).
  - Tables tab1/tab2: [102400, 128] bf16 (256 B rows; dma_gather requires
    256B-multiple rows; cols 32: are junk and never read by compute).
  - P1 sharded: each core computes its shard of x@W1 in bf16, AllGather
    (Shared DRAM) -> full tab1, then D2D-copied to Local DRAM per 32768-row
    window (gathers from Shared-space DRAM measured ~slower; window-aligned
    copies let window-w gathers start early).
  - Aggregation: flat edge stream per core ordered by (window, dst-tile),
    int16 idx per 32768-row window. dma_gather (mlp ucode lib, which DOES
    work on this runtime) with num_idxs=1024 (2048 crashes the device),
    round-robin across num_swdge_queues=4 (each SWDGE queue feeds ~1 DMA
    engine: 4 queues ~= 4x gather bandwidth - the single biggest lever).
    One-hot S matrices (DVE is_equal vs iota against compile-time lane ids,
    broadcast APs) x gathered rows on the PE accumulate per-dst sums in
    PSUM; pad slots get lane=255 (all-zero S row). Self-loops are plain
    edges. Layers 1+2 share one stream/idx/lane plan.
  - Runs (window,tile) padded to the max across cores so one program fits
    all 8 cores (~27% slot padding; the gather DMA is the bottleneck at
    ~23ns per 256B descriptor per engine, 4 engines).
  - BN folded into W2' = diag(s)@W2, c2 = t@W2 on host. relu/dinv/bias
    epilogue per tile; PE transpose + W2' matmul produces tab2 rows.

Measured pitfalls (this axon runtime): host wall-clock is quantized/noisy
(~40ms ticks + ms jitter) -> all timing via reps-differential paired deltas,
trimmed mean; engine instruction dispatch ~0.5-1.4us makes instruction
count the cost unit; dma_scatter_add loses concurrent duplicate-dst updates
(unusable for aggregation).
"""

import numpy as np
import ml_dtypes

import concourse.bass as bass
import concourse.bacc as bacc
import concourse.mybir as mybir
import concourse.tile as tile
from concourse.bass_utils import run_bass_kernel_spmd

F32 = mybir.dt.float32
BF16 = mybir.dt.bfloat16
I16 = mybir.dt.int16

C = 8
P = 128
H = 32
D = 512
NOCOPY = False
BN_EPS = 1e-5
WIN = 32768
BF = ml_dtypes.bfloat16


def _plan2(n_nodes, edge_index, K=1024):
    """Host graph preprocessing -> uniform stream plan + per-core data."""
    src = np.asarray(edge_index[0], dtype=np.int64)
    dst = np.asarray(edge_index[1], dtype=np.int64)

    deg = np.bincount(dst, minlength=n_nodes).astype(np.float32) + 1.0
    dinv = (1.0 / np.sqrt(deg)).astype(np.float32)

    per = n_nodes // C
    SH = -(-per // 512) * 512
    T = (per + P - 1) // P          # dst tiles per core (98)
    T_all = SH // P
    TAB = C * SH
    NW = -(-TAB // WIN)             # 4 windows

    order = np.argsort(deg, kind="stable")
    owner = np.empty(n_nodes, dtype=np.int64)
    pos = np.empty(n_nodes, dtype=np.int64)
    ranks = np.arange(n_nodes)
    owner[order] = ranks % C
    pos[order] = ranks // C
    row = owner * SH + pos

    # edges + self-loops
    es = np.concatenate([src, np.arange(n_nodes)])
    ed = np.concatenate([dst, np.arange(n_nodes)])
    eo = owner[ed]                  # owning core (by dst)
    ep = pos[ed]
    er = row[es]                    # table row of source
    et = ep // P                    # dst tile
    el = ep % P                     # dst lane
    ew = er // WIN                  # source window
    ei = (er - ew * WIN).astype(np.int64)  # int16 gather index

    # counts per (core, window, tile)
    key = (eo * NW + ew) * T + et
    cnt = np.bincount(key, minlength=C * NW * T).reshape(C, NW, T)
    cmax = cnt.max(axis=0)                      # [NW, T]
    rlen = -(-cmax // P) * P                    # padded run length, [NW, T]
    run_start = np.zeros((NW, T), np.int64)     # slot offset of each run
    flat = rlen.reshape(-1)
    starts = np.concatenate([[0], np.cumsum(flat)[:-1]])
    run_start = starts.reshape(NW, T)
    slots_tot = int(flat.sum())
    NCH = slots_tot // P

    # instructions: chop each window segment into <=K slot pieces
    wseg = rlen.sum(axis=1)                     # slots per window
    wstart = np.concatenate([[0], np.cumsum(wseg)[:-1]])
    instrs = []                                 # (w, slot0, nslots)
    for w in range(NW):
        s0, s1 = int(wstart[w]), int(wstart[w] + wseg[w])
        while s0 < s1:
            n = min(K, s1 - s0)
            instrs.append((w, s0, n))
            s0 += n
    # chunk -> instr index
    instr_slot0 = np.array([s for (_, s, _) in instrs])
    NI = len(instrs)

    # runs in stream order: (w, t, chunk0, nchunks)
    runs = []
    first_for_tile = {}
    for w in range(NW):
        for t in range(T):
            if rlen[w, t] == 0:
                continue
            ch0 = int(run_start[w, t]) // P
            nch = int(rlen[w, t]) // P
            first = t not in first_for_tile
            first_for_tile[t] = True
            runs.append((w, t, ch0, nch, first))

    # per-core idx + lane streams in the shared layout
    idx_flat = np.zeros((C, slots_tot), np.int16)
    lane_flat = np.full((C, slots_tot), 255, np.int16)
    eorder = np.lexsort((ei, et, ew, eo))
    so, sw, st_, sl, si = (eo[eorder], ew[eorder], et[eorder],
                           el[eorder], ei[eorder])
    grp = (so * NW + sw) * T + st_
    newgrp = np.ones(len(grp), dtype=bool)
    newgrp[1:] = grp[1:] != grp[:-1]
    gstart = np.where(newgrp)[0]
    within = np.arange(len(grp)) - np.repeat(
        gstart, np.diff(np.concatenate([gstart, [len(grp)]])))
    slot = run_start[sw, st_] + within
    idx_flat[so, slot] = si.astype(np.int16)
    lane_flat[so, slot] = sl.astype(np.int16)

    # wrapped int16 idx: per instruction, k -> [k%16, off + k//16]
    NW16 = slots_tot // 16
    idx_wrap = np.zeros((C, 16, NW16), np.int16)
    for (w, s0, n) in instrs:
        blk = idx_flat[:, s0:s0 + n].reshape(C, n // 16, 16)
        idx_wrap[:, :, s0 // 16:(s0 + n) // 16] = blk.transpose(0, 2, 1)
    idx_wrap = np.tile(idx_wrap, (1, 8, 1))     # replicate to 128 partitions

    # lanes in [128, NCH] chunk-column layout (bf16 on device)
    lanes = lane_flat.reshape(C, NCH, P).transpose(0, 2, 1)  # [C,128,NCH]

    # dinv in shard order per core
    nodes_by_cp = np.full((C, per), -1, dtype=np.int64)
    nodes_by_cp[owner, pos] = np.arange(n_nodes)
    dinv_s = np.zeros((C, P, T_all), np.float32)
    for c in range(C):
        fulls = np.zeros(SH, np.float32)
        fulls[:per] = dinv[nodes_by_cp[c]]
        dinv_s[c] = fulls.reshape(T_all, P).T

    meta = dict(per=per, SH=SH, T=T, T_all=T_all, TAB=TAB, NW=NW, K=K,
                NI=NI, NCH=NCH, NW16=NW16, slots_tot=slots_tot,
                instrs=instrs, runs=runs, nodes_by_cp=nodes_by_cp,
                dinv=dinv, wseg=wseg, wstart=wstart)
    data = dict(idx_wrap=idx_wrap, lanes=lanes, dinv_s=dinv_s,
                lane_flat=lane_flat, idx_flat=idx_flat)
    return meta, data


def _build_nc2(meta, phases=("p1", "ag1", "l1", "ag2", "l2"), reps=1,
               scratch=65536, tiny_out=False):
    phases = set(phases)
    SH, T, T_all, TAB = meta["SH"], meta["T"], meta["T_all"], meta["TAB"]
    NW, K, NI, NCH, NW16 = (meta["NW"], meta["K"], meta["NI"], meta["NCH"],
                            meta["NW16"])
    instrs, runs = meta["instrs"], meta["runs"]
    J = K // P

    nc = bacc.Bacc("TRN2", target_bir_lowering=False, debug=False,
                   num_devices=C, dynamic_dma_scratch_size=scratch,
                   num_swdge_queues=4)
    xT = nc.dram_tensor("xT", [D, SH], BF16, kind="ExternalInput").ap()
    w1 = nc.dram_tensor("w1", [D, H], BF16, kind="ExternalInput").ap()
    w2p = nc.dram_tensor("w2p", [H, H], F32, kind="ExternalInput").ap()
    b1r = nc.dram_tensor("b1r", [P, H], F32, kind="ExternalInput").ap()
    b2r = nc.dram_tensor("b2r", [P, H], F32, kind="ExternalInput").ap()
    c2r = nc.dram_tensor("c2r", [P, H], F32, kind="ExternalInput").ap()
    ident = nc.dram_tensor("ident", [P, P], F32, kind="ExternalInput").ap()
    iota = nc.dram_tensor("iota", [P, P], BF16, kind="ExternalInput").ap()
    dinvs = nc.dram_tensor("dinvs", [P, T_all], F32, kind="ExternalInput").ap()
    lanesd = nc.dram_tensor("lanes", [P, NCH], BF16, kind="ExternalInput").ap()
    idxd = nc.dram_tensor("idx", [P, NW16], I16, kind="ExternalInput").ap()
    out = nc.dram_tensor("out", [P, H] if tiny_out else [SH, H], F32,
                         kind="ExternalOutput").ap()

    with tile.TileContext(nc) as tc:
        with (
            tc.tile_pool(name="cst", bufs=1) as cst,
            tc.tile_pool(name="sb", bufs=3) as sb,
            tc.tile_pool(name="gp", bufs=4) as gp,
            tc.tile_pool(name="sp", bufs=4) as sp,
            tc.tile_pool(name="ps", bufs=2, space="PSUM") as ps,
            tc.tile_pool(name="pt", bufs=2, space="PSUM") as pt_pool,
            tc.tile_pool(name="dram", bufs=1, space="DRAM") as dram,
        ):
            tab1l = dram.tile([SH, P], BF16)
            h2l = dram.tile([SH, P], BF16)
            tab1c = dram.tile([TAB, P], BF16, name="tab1c")
            tab2c = dram.tile([TAB, P], BF16, name="tab2c")
            outd = dram.tile([SH, H], F32, name="outd") if tiny_out else out

            # constants
            w1t = cst.tile([P, 4 * H], BF16)
            for f in range(4):
                nc.sync.dma_start(w1t[:, f * H:(f + 1) * H],
                                  w1[f * P:(f + 1) * P, :])
            w2pt = cst.tile([H, H], F32)
            nc.sync.dma_start(w2pt[:], w2p[:, :])
            b1t = cst.tile([P, H], F32)
            nc.sync.dma_start(b1t[:], b1r[:, :])
            b2t = cst.tile([P, H], F32)
            nc.sync.dma_start(b2t[:], b2r[:, :])
            c2t = cst.tile([P, H], F32)
            nc.sync.dma_start(c2t[:], c2r[:, :])
            idt = cst.tile([P, P], F32)
            nc.sync.dma_start(idt[:], ident[:, :])
            iot = cst.tile([P, P], BF16)
            nc.sync.dma_start(iot[:], iota[:, :])
            dst_ = cst.tile([P, T_all], F32)
            nc.sync.dma_start(dst_[:], dinvs[:, :])
            lant = cst.tile([P, NCH], BF16)
            nc.sync.dma_start(lant[:], lanesd[:, :])
            ixt = cst.tile([P, NW16], I16)
            nc.sync.dma_start(ixt[:], idxd[:, :])
            acc = cst.tile([P, T * H], F32)

            env = dict(locals())
            for _rep in range(reps):
                tag = f"r{_rep}" if _rep else ""
                env["tab1"] = dram.tile([TAB, P], BF16, addr_space="Shared",
                                        name=f"tab1{tag}", tag=f"tab1{tag}")
                env["tab2"] = dram.tile([TAB, P], BF16, addr_space="Shared",
                                        name=f"tab2{tag}", tag=f"tab2{tag}")
                _body2(nc, tc, phases, meta, env)

    nc.compile()
    return nc


def _gather_layer(nc, meta, env, tab, which, lvl=2):
    """Issue gathers + S-gen + PE accumulation into acc for one layer."""
    instrs, runs = meta["instrs"], meta["runs"]
    K, NW = meta["K"], meta["NW"]
    TAB = meta["TAB"]
    J = K // P
    gp, sp, ps = env["gp"], env["sp"], env["ps"]
    ixt, lant, iot, acc = env["ixt"], env["lant"], env["iot"], env["acc"]

    instr_slot0 = [s for (_, s, _) in instrs]
    gtiles = {}
    stiles = {}

    def ensure_instr(i):
        if i in gtiles:
            return
        (w, s0, n) = instrs[i]
        jn = n // P
        wsz = min(WIN, TAB - w * WIN)
        g = gp.tile([P, J * P], BF16, tag="g", name="g")
        nc.gpsimd.dma_gather(
            g[:, :jn * P].rearrange("p (j f) -> p j f", f=P),
            tab[w * WIN: w * WIN + wsz, :],
            ixt[:, s0 // 16:(s0 + n) // 16],
            n, n, P, queue_num=i % 4)
        s = sp.tile([P, J * P], BF16, tag="s", name="s")
        ch0 = s0 // P
        if lvl >= 1:
            nc.vector.tensor_tensor(
                out=s[:, :jn * P].rearrange("p (j f) -> p j f", f=P),
                in0=lant[:, ch0:ch0 + jn].unsqueeze(2).broadcast_to((P, jn, P)),
                in1=iot[:].unsqueeze(1).broadcast_to((P, jn, P)),
                op=mybir.AluOpType.is_equal)
        gtiles[i] = g
        stiles[i] = s

    import bisect
    if lvl < 2:
        for i in range(len(instrs)):
            ensure_instr(i)
        return
    for (w, t, ch0, nch, first) in runs:
        pp = ps.tile([P, H], F32, tag="agg", name="pp")
        for k in range(nch):
            ch = ch0 + k
            i = bisect.bisect_right(instr_slot0, ch * P) - 1
            ensure_instr(i)
            lc = (ch * P - instr_slot0[i]) // P
            nc.tensor.matmul(
                pp[:],
                lhsT=stiles[i][:, lc * P:(lc + 1) * P],
                rhs=gtiles[i][:, lc * P:lc * P + H],
                start=(k == 0), stop=(k == nch - 1))
        if first:
            nc.scalar.activation(acc[:, t * H:(t + 1) * H], pp[:],
                                 mybir.ActivationFunctionType.Copy)
        else:
            nc.vector.tensor_add(acc[:, t * H:(t + 1) * H],
                                 acc[:, t * H:(t + 1) * H], pp[:])


def _body2(nc, tc, phases, meta, env):
    SH, T, T_all, TAB = meta["SH"], meta["T"], meta["T_all"], meta["TAB"]
    sb, ps, pt_pool = env["sb"], env["ps"], env["pt_pool"]
    xT, out = env["xT"], env["out"]
    outd = env["outd"]
    tab1c, tab2c = env["tab1c"], env["tab2c"]
    tab1l, h2l = env["tab1l"], env["h2l"]
    tab1, tab2 = env["tab1"], env["tab2"]
    w1t, w2pt = env["w1t"], env["w2pt"]
    b1t, b2t, c2t = env["b1t"], env["b2t"], env["c2t"]
    idt, dst_, acc = env["idt"], env["dst_"], env["acc"]

    NST = SH // 512
    # ---- P1: this core's shard of x @ W1, scaled by dinv ----
    if "p1" in phases:
        for st in range(NST):
            xt = sb.tile([P, 4 * D], BF16, tag="xt")
            for f in range(4):
                nc.sync.dma_start(
                    xt[:, f * D:(f + 1) * D],
                    xT[f * P:(f + 1) * P, st * 512:(st + 1) * 512])
            for g4 in range(4):
                pp = ps.tile([P, H], F32, tag="p1ps")
                for f in range(4):
                    nc.tensor.matmul(
                        pp[:],
                        lhsT=xt[:, f * D + g4 * P: f * D + (g4 + 1) * P],
                        rhs=w1t[:, f * H:(f + 1) * H],
                        start=(f == 0), stop=(f == 3))
                g = st * 4 + g4
                ht = sb.tile([P, P], BF16, tag="ht")
                nc.scalar.activation(ht[:, 0:H], pp[:],
                                     mybir.ActivationFunctionType.Copy,
                                     scale=dst_[:, g:g + 1])
                nc.sync.dma_start(tab1l[g * P:(g + 1) * P, :], ht[:])

    if "ag1" in phases:
        nc.gpsimd.collective_compute(
            "AllGather", mybir.AluOpType.bypass,
            replica_groups=[list(range(C))],
            ins=[tab1l.opt()], outs=[tab1.opt()])

    # ---- Layer 1: gather-accumulate + epilogue ----
    t1src = tab1 if NOCOPY else tab1c
    if not NOCOPY and phases & {"l1", "l1g", "l1ge"}:
        for w in range(4):
            base = w * WIN
            wsz = min(WIN, TAB - base)
            for h in range(2):
                hs = wsz // 2 if wsz > WIN // 2 else wsz
                if h * (wsz // 2) >= wsz:
                    continue
                lo = base + h * (wsz // 2)
                hi = min(base + wsz, lo + max(wsz // 2, 1))
                if h == 1:
                    hi = base + wsz
                nc.sync.dma_start(tab1c[lo:hi, :], tab1[lo:hi, :])
    if "l1g" in phases:
        _gather_layer(nc, meta, env, t1src, "a", lvl=0)
    if "l1ge" in phases:
        _gather_layer(nc, meta, env, t1src, "a", lvl=1)
    if "l1" in phases:
        _gather_layer(nc, meta, env, t1src, "a")
        for t in range(T):
            red = sb.tile([P, H], F32, tag="red")
            nc.vector.tensor_scalar_mul(red[:], acc[:, t * H:(t + 1) * H],
                                        dst_[:, t:t + 1])
            nc.vector.tensor_add(red[:], red[:], b1t[:])
            nc.vector.tensor_scalar_max(red[:], red[:], 0.0)
            pt = pt_pool.tile([H, P], F32, tag="pst")
            nc.tensor.transpose(pt[:], red[:], idt[:])
            rt = sb.tile([H, P], F32, tag="rt")
            nc.scalar.activation(rt[:], pt[:],
                                 mybir.ActivationFunctionType.Copy)
            p2 = pt_pool.tile([P, H], F32, tag="ps2")
            nc.tensor.matmul(p2[:], lhsT=rt[:], rhs=w2pt[:],
                             start=True, stop=True)
            h2f = sb.tile([P, H], F32, tag="h2f")
            nc.vector.tensor_add(h2f[:], p2[:], c2t[:])
            h2t = sb.tile([P, P], BF16, tag="h2t")
            nc.vector.tensor_scalar_mul(h2t[:, 0:H], h2f[:],
                                        dst_[:, t:t + 1])
            nc.sync.dma_start(h2l[t * P:(t + 1) * P, :], h2t[:])

    if "ag2" in phases:
        nc.gpsimd.collective_compute(
            "AllGather", mybir.AluOpType.bypass,
            replica_groups=[list(range(C))],
            ins=[h2l.opt()], outs=[tab2.opt()])

    # ---- liveness touch: copy a slice of the deepest phase's output ----
    if "l2" not in phases:
        deep = None
        if "ag2" in phases:
            deep = tab2
        elif "l1" in phases:
            deep = h2l
        elif "ag1" in phases:
            deep = tab1
        elif "p1" in phases:
            deep = tab1l
        if deep is not None:
            tch = sb.tile([P, H], BF16, tag="tch", name="tch")
            nc.sync.dma_start(tch[:], deep[0:P, 0:H])
            tchf = sb.tile([P, H], F32, tag="tchf", name="tchf")
            nc.vector.tensor_copy(tchf[:], tch[:])
            nc.sync.dma_start(out[0:P, :], tchf[:])

    # ---- Layer 2 ----
    if "l2" in phases:
        if not NOCOPY:
            for w in range(4):
                base = w * WIN
                wsz = min(WIN, TAB - base)
                for h in range(2):
                    if h * (wsz // 2) >= wsz:
                        continue
                    lo = base + h * (wsz // 2)
                    hi = base + wsz if h == 1 else lo + wsz // 2
                    nc.sync.dma_start(tab2c[lo:hi, :], tab2[lo:hi, :])
        _gather_layer(nc, meta, env, tab2 if NOCOPY else tab2c, "b")
        for t in range(T):
            red = sb.tile([P, H], F32, tag="red2")
            nc.vector.tensor_scalar_mul(red[:], acc[:, t * H:(t + 1) * H],
                                        dst_[:, t:t + 1])
            nc.vector.tensor_add(red[:], red[:], b2t[:])
            ot = sb.tile([P, H], F32, tag="ot")
            nc.vector.tensor_scalar_max(ot[:], red[:], 0.0)
            nc.sync.dma_start(outd[t * P:(t + 1) * P, :], ot[:])
        if outd is not out:
            tc2 = sb.tile([P, H], F32, tag="tc2", name="tc2")
            nc.sync.dma_start(tc2[:], outd[0:P, :])
            nc.sync.dma_start(out[0:P, :], tc2[:])


def _impl2(x, edge_index, W1, b1, W2, b2, gamma, beta, run_mean, run_var,
           n_nodes):
    x = np.asarray(x, np.float32)
    W1 = np.asarray(W1, np.float32)
    b1 = np.asarray(b1, np.float32)
    W2 = np.asarray(W2, np.float32)
    b2 = np.asarray(b2, np.float32)
    gamma = np.asarray(gamma, np.float32)
    beta = np.asarray(beta, np.float32)
    run_mean = np.asarray(run_mean, np.float32)
    run_var = np.asarray(run_var, np.float32)

    meta, data = _plan2(n_nodes, np.asarray(edge_index))
    per, SH, T_all = meta["per"], meta["SH"], meta["T_all"]

    s = gamma / np.sqrt(run_var + BN_EPS)
    t = beta - run_mean * s
    W2p = (W2 * s[:, None]).astype(np.float32)
    c2 = (t @ W2).astype(np.float32)

    b1rep = np.tile(b1[None, :], (P, 1)).astype(np.float32)
    b2rep = np.tile(b2[None, :], (P, 1)).astype(np.float32)
    c2rep = np.tile(c2[None, :], (P, 1)).astype(np.float32)
    identv = np.eye(P, dtype=np.float32)
    iotav = np.tile(np.arange(P, dtype=np.float32)[None, :],
                    (P, 1)).astype(BF)

    nodes_by_cp = meta["nodes_by_cp"]
    in_maps = []
    for c in range(C):
        xs = np.zeros((SH, D), np.float32)
        xs[:per] = x[nodes_by_cp[c]]
        in_maps.append({
            "xT": np.ascontiguousarray(xs.T).astype(BF),
            "w1": W1.astype(BF), "w2p": W2p,
            "b1r": b1rep, "b2r": b2rep, "c2r": c2rep,
            "ident": identv, "iota": iotav,
            "dinvs": np.ascontiguousarray(data["dinv_s"][c]),
            "lanes": np.ascontiguousarray(data["lanes"][c]).astype(BF),
            "idx": np.ascontiguousarray(data["idx_wrap"][c]),
        })

    nc = _build_nc2(meta)
    global _LAST_NC, _LAST_IN_MAPS, _LAST_META
    _LAST_NC, _LAST_IN_MAPS, _LAST_META = nc, in_maps, meta
    res = run_bass_kernel_spmd(nc, in_maps, core_ids=list(range(C))).results

    outf = np.zeros((n_nodes, H), np.float32)
    for c in range(C):
        outf[nodes_by_cp[c]] = res[c]["out"][:per]
    return outf


def kernel(x, edge_index, W1, b1, W2, b2, gamma, beta, run_mean, run_var):
    return _impl2(x, edge_index, W1, b1, W2, b2, gamma, beta, run_mean,
                  run_var, n_nodes=100000)


# revision 5
# speedup vs baseline: 1.5570x; 1.5570x over previous
"""2-layer GCN on 8 trn2 cores — dma_gather (ucode) + one-hot matmul reduce.

Design (single SPMD program, ~3.3 ms vs 6.77 ms indirect-DMA baseline):
  - Nodes ranked by degree, dealt round-robin to cores. Table row of node n:
    row = owner*SH + pos (SH=12800, TAB=102400).
  - Tables tab1/tab2: [102400, 128] bf16 (256 B rows; dma_gather requires
    256B-multiple rows; cols 32: are junk and never read by compute).
  - P1 sharded: each core computes its shard of x@W1 in bf16, AllGather
    (Shared DRAM) -> full tab1, then D2D-copied to Local DRAM per 32768-row
    window (gathers from Shared-space DRAM measured ~slower; window-aligned
    copies let window-w gathers start early).
  - Aggregation: flat edge stream per core ordered by (window, dst-tile),
    int16 idx per 32768-row window. dma_gather (mlp ucode lib, which DOES
    work on this runtime) with num_idxs=1024 (2048 crashes the device),
    round-robin across num_swdge_queues=4 (each SWDGE queue feeds ~1 DMA
    engine: 4 queues ~= 4x gather bandwidth - the single biggest lever).
    One-hot S matrices (DVE is_equal vs iota against compile-time lane ids,
    broadcast APs) x gathered rows on the PE accumulate per-dst sums in
    PSUM; pad slots get lane=255 (all-zero S row). Self-loops are plain
    edges. Layers 1+2 share one stream/idx/lane plan.
  - Runs (window,tile) padded to the max across cores so one program fits
    all 8 cores (~27% slot padding; the gather DMA is the bottleneck at
    ~23ns per 256B descriptor per engine, 4 engines).
  - BN folded into W2' = diag(s)@W2, c2 = t@W2 on host. relu/dinv/bias
    epilogue per tile; PE transpose + W2' matmul produces tab2 rows.

Measured pitfalls (this axon runtime): host wall-clock is quantized/noisy
(~40ms ticks + ms jitter) -> all timing via reps-differential paired deltas,
trimmed mean; engine instruction dispatch ~0.5-1.4us makes instruction
count the cost unit; dma_scatter_add loses concurrent duplicate-dst updates
(unusable for aggregation).
"""

import numpy as np
import ml_dtypes

import concourse.bass as bass
import concourse.bacc as bacc
import concourse.mybir as mybir
import concourse.tile as tile
from concourse.bass_utils import run_bass_kernel_spmd

F32 = mybir.dt.float32
BF16 = mybir.dt.bfloat16
I16 = mybir.dt.int16

C = 8
P = 128
H = 32
D = 512
NOCOPY = False
BN_EPS = 1e-5
WIN = 32768
BF = ml_dtypes.bfloat16


def _plan2(n_nodes, edge_index, K=1024):
    """Host graph preprocessing -> uniform stream plan + per-core data."""
    src = np.asarray(edge_index[0], dtype=np.int64)
    dst = np.asarray(edge_index[1], dtype=np.int64)

    deg = np.bincount(dst, minlength=n_nodes).astype(np.float32) + 1.0
    dinv = (1.0 / np.sqrt(deg)).astype(np.float32)

    per = n_nodes // C
    SH = -(-per // 512) * 512
    T = (per + P - 1) // P          # dst tiles per core (98)
    T_all = SH // P
    TAB = C * SH
    NW = -(-TAB // WIN)             # 4 windows

    order = np.argsort(deg, kind="stable")
    owner = np.empty(n_nodes, dtype=np.int64)
    pos = np.empty(n_nodes, dtype=np.int64)
    ranks = np.arange(n_nodes)
    owner[order] = ranks % C
    pos[order] = ranks // C
    row = owner * SH + pos

    # edges + self-loops
    es = np.concatenate([src, np.arange(n_nodes)])
    ed = np.concatenate([dst, np.arange(n_nodes)])
    eo = owner[ed]                  # owning core (by dst)
    ep = pos[ed]
    er = row[es]                    # table row of source
    et = ep // P                    # dst tile
    el = ep % P                     # dst lane
    ew = er // WIN                  # source window
    ei = (er - ew * WIN).astype(np.int64)  # int16 gather index

    # counts per (core, window, tile)
    key = (eo * NW + ew) * T + et
    cnt = np.bincount(key, minlength=C * NW * T).reshape(C, NW, T)
    cmax = cnt.max(axis=0)                      # [NW, T]
    rlen = -(-cmax // P) * P                    # padded run length, [NW, T]
    run_start = np.zeros((NW, T), np.int64)     # slot offset of each run
    flat = rlen.reshape(-1)
    starts = np.concatenate([[0], np.cumsum(flat)[:-1]])
    run_start = starts.reshape(NW, T)
    slots_tot = int(flat.sum())
    NCH = slots_tot // P

    # instructions: chop each window segment into <=K slot pieces
    wseg = rlen.sum(axis=1)                     # slots per window
    wstart = np.concatenate([[0], np.cumsum(wseg)[:-1]])
    instrs = []                                 # (w, slot0, nslots)
    for w in range(NW):
        s0, s1 = int(wstart[w]), int(wstart[w] + wseg[w])
        while s0 < s1:
            n = min(K, s1 - s0)
            instrs.append((w, s0, n))
            s0 += n
    # chunk -> instr index
    instr_slot0 = np.array([s for (_, s, _) in instrs])
    NI = len(instrs)

    # runs in stream order: (w, t, chunk0, nchunks)
    runs = []
    first_for_tile = {}
    for w in range(NW):
        for t in range(T):
            if rlen[w, t] == 0:
                continue
            ch0 = int(run_start[w, t]) // P
            nch = int(rlen[w, t]) // P
            first = t not in first_for_tile
            first_for_tile[t] = True
            runs.append((w, t, ch0, nch, first))

    # per-core idx + lane streams in the shared layout
    idx_flat = np.zeros((C, slots_tot), np.int16)
    lane_flat = np.full((C, slots_tot), 255, np.int16)
    eorder = np.lexsort((ei, et, ew, eo))
    so, sw, st_, sl, si = (eo[eorder], ew[eorder], et[eorder],
                           el[eorder], ei[eorder])
    grp = (so * NW + sw) * T + st_
    newgrp = np.ones(len(grp), dtype=bool)
    newgrp[1:] = grp[1:] != grp[:-1]
    gstart = np.where(newgrp)[0]
    within = np.arange(len(grp)) - np.repeat(
        gstart, np.diff(np.concatenate([gstart, [len(grp)]])))
    slot = run_start[sw, st_] + within
    idx_flat[so, slot] = si.astype(np.int16)
    lane_flat[so, slot] = sl.astype(np.int16)

    # wrapped int16 idx: per instruction, k -> [k%16, off + k//16]
    NW16 = slots_tot // 16
    idx_wrap = np.zeros((C, 16, NW16), np.int16)
    for (w, s0, n) in instrs:
        blk = idx_flat[:, s0:s0 + n].reshape(C, n // 16, 16)
        idx_wrap[:, :, s0 // 16:(s0 + n) // 16] = blk.transpose(0, 2, 1)
    idx_wrap = np.tile(idx_wrap, (1, 8, 1))     # replicate to 128 partitions

    # lanes in [128, NCH] chunk-column layout (bf16 on device)
    lanes = lane_flat.reshape(C, NCH, P).transpose(0, 2, 1)  # [C,128,NCH]

    # dinv in shard order per core
    nodes_by_cp = np.full((C, per), -1, dtype=np.int64)
    nodes_by_cp[owner, pos] = np.arange(n_nodes)
    dinv_s = np.zeros((C, P, T_all), np.float32)
    for c in range(C):
        fulls = np.zeros(SH, np.float32)
        fulls[:per] = dinv[nodes_by_cp[c]]
        dinv_s[c] = fulls.reshape(T_all, P).T

    meta = dict(per=per, SH=SH, T=T, T_all=T_all, TAB=TAB, NW=NW, K=K,
                NI=NI, NCH=NCH, NW16=NW16, slots_tot=slots_tot,
                instrs=instrs, runs=runs, nodes_by_cp=nodes_by_cp,
                dinv=dinv, wseg=wseg, wstart=wstart)
    data = dict(idx_wrap=idx_wrap, lanes=lanes, dinv_s=dinv_s,
                lane_flat=lane_flat, idx_flat=idx_flat)
    return meta, data


def _build_nc2(meta, phases=("p1", "ag1", "l1", "ag2", "l2"), reps=1,
               scratch=65536, tiny_out=False):
    phases = set(phases)
    SH, T, T_all, TAB = meta["SH"], meta["T"], meta["T_all"], meta["TAB"]
    NW, K, NI, NCH, NW16 = (meta["NW"], meta["K"], meta["NI"], meta["NCH"],
                            meta["NW16"])
    instrs, runs = meta["instrs"], meta["runs"]
    J = K // P

    nc = bacc.Bacc("TRN2", target_bir_lowering=False, debug=False,
                   num_devices=C, dynamic_dma_scratch_size=scratch,
                   num_swdge_queues=4)
    xT = nc.dram_tensor("xT", [D, SH], BF16, kind="ExternalInput").ap()
    w1 = nc.dram_tensor("w1", [D, H], BF16, kind="ExternalInput").ap()
    w2p = nc.dram_tensor("w2p", [H, H], F32, kind="ExternalInput").ap()
    b1r = nc.dram_tensor("b1r", [P, H], F32, kind="ExternalInput").ap()
    b2r = nc.dram_tensor("b2r", [P, H], F32, kind="ExternalInput").ap()
    c2r = nc.dram_tensor("c2r", [P, H], F32, kind="ExternalInput").ap()
    ident = nc.dram_tensor("ident", [P, P], F32, kind="ExternalInput").ap()
    iota = nc.dram_tensor("iota", [P, P], BF16, kind="ExternalInput").ap()
    dinvs = nc.dram_tensor("dinvs", [P, T_all], F32, kind="ExternalInput").ap()
    lanesd = nc.dram_tensor("lanes", [P, NCH], BF16, kind="ExternalInput").ap()
    idxd = nc.dram_tensor("idx", [P, NW16], I16, kind="ExternalInput").ap()
    out = nc.dram_tensor("out", [P, H] if tiny_out else [SH, H], F32,
                         kind="ExternalOutput").ap()

    with tile.TileContext(nc) as tc:
        with (
            tc.tile_pool(name="cst", bufs=1) as cst,
            tc.tile_pool(name="sb", bufs=3) as sb,
            tc.tile_pool(name="gp", bufs=4) as gp,
            tc.tile_pool(name="sp", bufs=4) as sp,
            tc.tile_pool(name="ps", bufs=2, space="PSUM") as ps,
            tc.tile_pool(name="pt", bufs=2, space="PSUM") as pt_pool,
            tc.tile_pool(name="dram", bufs=1, space="DRAM") as dram,
        ):
            tab1l = dram.tile([SH, P], BF16)
            h2l = dram.tile([SH, P], BF16)
            tab1c = dram.tile([TAB, P], BF16, name="tab1c")
            tab2c = dram.tile([TAB, P], BF16, name="tab2c")
            outd = dram.tile([SH, H], F32, name="outd") if tiny_out else out

            # constants
            w1t = cst.tile([P, 4 * H], BF16)
            for f in range(4):
                nc.sync.dma_start(w1t[:, f * H:(f + 1) * H],
                                  w1[f * P:(f + 1) * P, :])
            w2pt = cst.tile([H, H], F32)
            nc.sync.dma_start(w2pt[:], w2p[:, :])
            b1t = cst.tile([P, H], F32)
            nc.sync.dma_start(b1t[:], b1r[:, :])
            b2t = cst.tile([P, H], F32)
            nc.sync.dma_start(b2t[:], b2r[:, :])
            c2t = cst.tile([P, H], F32)
            nc.sync.dma_start(c2t[:], c2r[:, :])
            idt = cst.tile([P, P], F32)
            nc.sync.dma_start(idt[:], ident[:, :])
            iot = cst.tile([P, P], BF16)
            nc.sync.dma_start(iot[:], iota[:, :])
            dst_ = cst.tile([P, T_all], F32)
            nc.sync.dma_start(dst_[:], dinvs[:, :])
            lant = cst.tile([P, NCH], BF16)
            nc.sync.dma_start(lant[:], lanesd[:, :])
            ixt = cst.tile([P, NW16], I16)
            nc.sync.dma_start(ixt[:], idxd[:, :])
            acc = cst.tile([P, T * H], F32)

            env = dict(locals())
            for _rep in range(reps):
                tag = f"r{_rep}" if _rep else ""
                env["tab1"] = dram.tile([TAB, P], BF16, addr_space="Shared",
                                        name=f"tab1{tag}", tag=f"tab1{tag}")
                env["tab2"] = dram.tile([TAB, P], BF16, addr_space="Shared",
                                        name=f"tab2{tag}", tag=f"tab2{tag}")
                _body2(nc, tc, phases, meta, env)

    nc.compile()
    return nc


def _gather_layer(nc, meta, env, tab, which, lvl=2):
    """Issue gathers + S-gen + PE accumulation into acc for one layer."""
    instrs, runs = meta["instrs"], meta["runs"]
    K, NW = meta["K"], meta["NW"]
    TAB = meta["TAB"]
    J = K // P
    gp, sp, ps = env["gp"], env["sp"], env["ps"]
    ixt, lant, iot, acc = env["ixt"], env["lant"], env["iot"], env["acc"]

    instr_slot0 = [s for (_, s, _) in instrs]
    gtiles = {}
    stiles = {}

    def ensure_instr(i):
        if i in gtiles:
            return
        (w, s0, n) = instrs[i]
        jn = n // P
        wsz = min(WIN, TAB - w * WIN)
        g = gp.tile([P, J * P], BF16, tag="g", name="g")
        nc.gpsimd.dma_gather(
            g[:, :jn * P].rearrange("p (j f) -> p j f", f=P),
            tab[w * WIN: w * WIN + wsz, :],
            ixt[:, s0 // 16:(s0 + n) // 16],
            n, n, P, queue_num=i % 4)
        s = sp.tile([P, J * P], BF16, tag="s", name="s")
        ch0 = s0 // P
        if lvl >= 1:
            nc.vector.tensor_tensor(
                out=s[:, :jn * P].rearrange("p (j f) -> p j f", f=P),
                in0=lant[:, ch0:ch0 + jn].unsqueeze(2).broadcast_to((P, jn, P)),
                in1=iot[:].unsqueeze(1).broadcast_to((P, jn, P)),
                op=mybir.AluOpType.is_equal)
        gtiles[i] = g
        stiles[i] = s

    import bisect
    if lvl < 2:
        for i in range(len(instrs)):
            ensure_instr(i)
        return
    for (w, t, ch0, nch, first) in runs:
        pp = ps.tile([P, H], F32, tag="agg", name="pp")
        for k in range(nch):
            ch = ch0 + k
            i = bisect.bisect_right(instr_slot0, ch * P) - 1
            ensure_instr(i)
            lc = (ch * P - instr_slot0[i]) // P
            nc.tensor.matmul(
                pp[:],
                lhsT=stiles[i][:, lc * P:(lc + 1) * P],
                rhs=gtiles[i][:, lc * P:lc * P + H],
                start=(k == 0), stop=(k == nch - 1))
        if first:
            nc.scalar.activation(acc[:, t * H:(t + 1) * H], pp[:],
                                 mybir.ActivationFunctionType.Copy)
        else:
            nc.vector.tensor_add(acc[:, t * H:(t + 1) * H],
                                 acc[:, t * H:(t + 1) * H], pp[:])


def _body2(nc, tc, phases, meta, env):
    SH, T, T_all, TAB = meta["SH"], meta["T"], meta["T_all"], meta["TAB"]
    sb, ps, pt_pool = env["sb"], env["ps"], env["pt_pool"]
    xT, out = env["xT"], env["out"]
    outd = env["outd"]
    tab1c, tab2c = env["tab1c"], env["tab2c"]
    tab1l, h2l = env["tab1l"], env["h2l"]
    tab1, tab2 = env["tab1"], env["tab2"]
    w1t, w2pt = env["w1t"], env["w2pt"]
    b1t, b2t, c2t = env["b1t"], env["b2t"], env["c2t"]
    idt, dst_, acc = env["idt"], env["dst_"], env["acc"]

    NST = SH // 512
    # ---- P1: this core's shard of x @ W1, scaled by dinv ----
    if "p1" in phases:
        for st in range(NST):
            xt = sb.tile([P, 4 * D], BF16, tag="xt")
            for f in range(4):
                nc.sync.dma_start(
                    xt[:, f * D:(f + 1) * D],
                    xT[f * P:(f + 1) * P, st * 512:(st + 1) * 512])
            for g4 in range(4):
                pp = ps.tile([P, H], F32, tag="p1ps")
                for f in range(4):
                    nc.tensor.matmul(
                        pp[:],
                        lhsT=xt[:, f * D + g4 * P: f * D + (g4 + 1) * P],
                        rhs=w1t[:, f * H:(f + 1) * H],
                        start=(f == 0), stop=(f == 3))
                g = st * 4 + g4
                ht = sb.tile([P, P], BF16, tag="ht")
                nc.scalar.activation(ht[:, 0:H], pp[:],
                                     mybir.ActivationFunctionType.Copy,
                                     scale=dst_[:, g:g + 1])
                nc.sync.dma_start(tab1l[g * P:(g + 1) * P, :], ht[:])

    if "ag1" in phases:
        nc.gpsimd.collective_compute(
            "AllGather", mybir.AluOpType.bypass,
            replica_groups=[list(range(C))],
            ins=[tab1l.opt()], outs=[tab1.opt()])

    # ---- Layer 1: gather-accumulate + epilogue ----
    t1src = tab1 if NOCOPY else tab1c
    if not NOCOPY and phases & {"l1", "l1g", "l1ge"}:
        for w in range(4):
            base = w * WIN
            wsz = min(WIN, TAB - base)
            for h in range(2):
                hs = wsz // 2 if wsz > WIN // 2 else wsz
                if h * (wsz // 2) >= wsz:
                    continue
                lo = base + h * (wsz // 2)
                hi = min(base + wsz, lo + max(wsz // 2, 1))
                if h == 1:
                    hi = base + wsz
                nc.sync.dma_start(tab1c[lo:hi, :], tab1[lo:hi, :])
    if "l1g" in phases:
        _gather_layer(nc, meta, env, t1src, "a", lvl=0)
    if "l1ge" in phases:
        _gather_layer(nc, meta, env, t1src, "a", lvl=1)
    if "l1" in phases:
        _gather_layer(nc, meta, env, t1src, "a")
        for t in range(T):
            red = sb.tile([P, H], F32, tag="red")
            nc.vector.tensor_scalar_mul(red[:], acc[:, t * H:(t + 1) * H],
                                        dst_[:, t:t + 1])
            nc.vector.tensor_add(red[:], red[:], b1t[:])
            nc.vector.tensor_scalar_max(red[:], red[:], 0.0)
            pt = pt_pool.tile([H, P], F32, tag="pst")
            nc.tensor.transpose(pt[:], red[:], idt[:])
            rt = sb.tile([H, P], F32, tag="rt")
            nc.scalar.activation(rt[:], pt[:],
                                 mybir.ActivationFunctionType.Copy)
            p2 = pt_pool.tile([P, H], F32, tag="ps2")
            nc.tensor.matmul(p2[:], lhsT=rt[:], rhs=w2pt[:],
                             start=True, stop=True)
            h2f = sb.tile([P, H], F32, tag="h2f")
            nc.vector.tensor_add(h2f[:], p2[:], c2t[:])
            h2t = sb.tile([P, P], BF16, tag="h2t")
            nc.vector.tensor_scalar_mul(h2t[:, 0:H], h2f[:],
                                        dst_[:, t:t + 1])
            nc.sync.dma_start(h2l[t * P:(t + 1) * P, :], h2t[:])

    if "ag2" in phases:
        nc.gpsimd.collective_compute(
            "AllGather", mybir.AluOpType.bypass,
            replica_groups=[list(range(C))],
            ins=[h2l.opt()], outs=[tab2.opt()])

    # ---- liveness touch: copy a slice of the deepest phase's output ----
    if "l2" not in phases:
        deep = None
        if "ag2" in phases:
            deep = tab2
        elif "l1" in phases:
            deep = h2l
        elif "ag1" in phases:
            deep = tab1
        elif "p1" in phases:
            deep = tab1l
        if deep is not None:
            tch = sb.tile([P, H], BF16, tag="tch", name="tch")
            nc.sync.dma_start(tch[:], deep[0:P, 0:H])
            tchf = sb.tile([P, H], F32, tag="tchf", name="tchf")
            nc.vector.tensor_copy(tchf[:], tch[:])
            nc.sync.dma_start(out[0:P, :], tchf[:])

    # ---- Layer 2 ----
    if "l2" in phases:
        if not NOCOPY:
            for w in range(4):
                base = w * WIN
                wsz = min(WIN, TAB - base)
                for h in range(2):
                    if h * (wsz // 2) >= wsz:
                        continue
                    lo = base + h * (wsz // 2)
                    hi = base + wsz if h == 1 else lo + wsz // 2
                    nc.sync.dma_start(tab2c[lo:hi, :], tab2[lo:hi, :])
        _gather_layer(nc, meta, env, tab2 if NOCOPY else tab2c, "b")
        for t in range(T):
            red = sb.tile([P, H], F32, tag="red2")
            nc.vector.tensor_scalar_mul(red[:], acc[:, t * H:(t + 1) * H],
                                        dst_[:, t:t + 1])
            nc.vector.tensor_add(red[:], red[:], b2t[:])
            ot = sb.tile([P, H], F32, tag="ot")
            nc.vector.tensor_scalar_max(ot[:], red[:], 0.0)
            nc.sync.dma_start(outd[t * P:(t + 1) * P, :], ot[:])
        if outd is not out:
            tc2 = sb.tile([P, H], F32, tag="tc2", name="tc2")
            nc.sync.dma_start(tc2[:], outd[0:P, :])
            nc.sync.dma_start(out[0:P, :], tc2[:])


def _impl2(x, edge_index, W1, b1, W2, b2, gamma, beta, run_mean, run_var,
           n_nodes):
    x = np.asarray(x, np.float32)
    W1 = np.asarray(W1, np.float32)
    b1 = np.asarray(b1, np.float32)
    W2 = np.asarray(W2, np.float32)
    b2 = np.asarray(b2, np.float32)
    gamma = np.asarray(gamma, np.float32)
    beta = np.asarray(beta, np.float32)
    run_mean = np.asarray(run_mean, np.float32)
    run_var = np.asarray(run_var, np.float32)

    meta, data = _plan2(n_nodes, np.asarray(edge_index))
    per, SH, T_all = meta["per"], meta["SH"], meta["T_all"]

    s = gamma / np.sqrt(run_var + BN_EPS)
    t = beta - run_mean * s
    W2p = (W2 * s[:, None]).astype(np.float32)
    c2 = (t @ W2).astype(np.float32)

    b1rep = np.tile(b1[None, :], (P, 1)).astype(np.float32)
    b2rep = np.tile(b2[None, :], (P, 1)).astype(np.float32)
    c2rep = np.tile(c2[None, :], (P, 1)).astype(np.float32)
    identv = np.eye(P, dtype=np.float32)
    iotav = np.tile(np.arange(P, dtype=np.float32)[None, :],
                    (P, 1)).astype(BF)

    nodes_by_cp = meta["nodes_by_cp"]
    in_maps = []
    for c in range(C):
        xs = np.zeros((SH, D), np.float32)
        xs[:per] = x[nodes_by_cp[c]]
        in_maps.append({
            "xT": np.ascontiguousarray(xs.T).astype(BF),
            "w1": W1.astype(BF), "w2p": W2p,
            "b1r": b1rep, "b2r": b2rep, "c2r": c2rep,
            "ident": identv, "iota": iotav,
            "dinvs": np.ascontiguousarray(data["dinv_s"][c]),
            "lanes": np.ascontiguousarray(data["lanes"][c]).astype(BF),
            "idx": np.ascontiguousarray(data["idx_wrap"][c]),
        })

    nc = _build_nc2(meta)
    global _LAST_NC, _LAST_IN_MAPS, _LAST_META
    _LAST_NC, _LAST_IN_MAPS, _LAST_META = nc, in_maps, meta
    res = run_bass_kernel_spmd(nc, in_maps, core_ids=list(range(C))).results

    outf = np.zeros((n_nodes, H), np.float32)
    for c in range(C):
        outf[nodes_by_cp[c]] = res[c]["out"][:per]
    return outf


def kernel(x, edge_index, W1, b1, W2, b2, gamma, beta, run_mean, run_var):
    return _impl2(x, edge_index, W1, b1, W2, b2, gamma, beta, run_mean,
                  run_var, n_nodes=100000)


# revision 6
# speedup vs baseline: 1.5598x; 1.0018x over previous
"""2-layer GCN on 8 trn2 cores — dma_gather (ucode) + one-hot matmul reduce.

Design (single SPMD program, ~3.3 ms vs 6.77 ms indirect-DMA baseline):
  - Nodes ranked by degree, dealt round-robin to cores. Table row of node n:
    row = owner*SH + pos (SH=12800, TAB=102400).
  - Tables tab1/tab2: [102400, 128] bf16 (256 B rows; dma_gather requires
    256B-multiple rows; cols 32: are junk and never read by compute).
  - P1 sharded: each core computes its shard of x@W1 in bf16, AllGather
    (Shared DRAM) -> full tab1, then D2D-copied to Local DRAM per 32768-row
    window (gathers from Shared-space DRAM measured ~slower; window-aligned
    copies let window-w gathers start early).
  - Aggregation: flat edge stream per core ordered by (window, dst-tile),
    int16 idx per 32768-row window. dma_gather (mlp ucode lib, which DOES
    work on this runtime) with num_idxs=1024 (2048 crashes the device),
    round-robin across num_swdge_queues=4 (each SWDGE queue feeds ~1 DMA
    engine: 4 queues ~= 4x gather bandwidth - the single biggest lever).
    One-hot S matrices (DVE is_equal vs iota against compile-time lane ids,
    broadcast APs) x gathered rows on the PE accumulate per-dst sums in
    PSUM; pad slots get lane=255 (all-zero S row). Self-loops are plain
    edges. Layers 1+2 share one stream/idx/lane plan.
  - Runs (window,tile) padded to the max across cores so one program fits
    all 8 cores (~27% slot padding; the gather DMA is the bottleneck at
    ~23ns per 256B descriptor per engine, 4 engines).
  - BN folded into W2' = diag(s)@W2, c2 = t@W2 on host. relu/dinv/bias
    epilogue per tile; PE transpose + W2' matmul produces tab2 rows.

Measured pitfalls (this axon runtime): host wall-clock is quantized/noisy
(~40ms ticks + ms jitter) -> all timing via reps-differential paired deltas,
trimmed mean; engine instruction dispatch ~0.5-1.4us makes instruction
count the cost unit; dma_scatter_add loses concurrent duplicate-dst updates
(unusable for aggregation).
"""

import numpy as np
import ml_dtypes

import concourse.bass as bass
import concourse.bacc as bacc
import concourse.mybir as mybir
import concourse.tile as tile
from concourse.bass_utils import run_bass_kernel_spmd

F32 = mybir.dt.float32
BF16 = mybir.dt.bfloat16
I16 = mybir.dt.int16

C = 8
P = 128
H = 32
D = 512
NOCOPY = False
BN_EPS = 1e-5
WIN = 32768
BF = ml_dtypes.bfloat16


def _plan2(n_nodes, edge_index, K=1024):
    """Host graph preprocessing -> uniform stream plan + per-core data."""
    src = np.asarray(edge_index[0], dtype=np.int64)
    dst = np.asarray(edge_index[1], dtype=np.int64)

    deg = np.bincount(dst, minlength=n_nodes).astype(np.float32) + 1.0
    dinv = (1.0 / np.sqrt(deg)).astype(np.float32)

    per = n_nodes // C
    SH = -(-per // 512) * 512
    T = (per + P - 1) // P          # dst tiles per core (98)
    T_all = SH // P
    TAB = C * SH
    NW = -(-TAB // WIN)             # 4 windows

    order = np.argsort(deg, kind="stable")
    owner = np.empty(n_nodes, dtype=np.int64)
    pos = np.empty(n_nodes, dtype=np.int64)
    ranks = np.arange(n_nodes)
    owner[order] = ranks % C
    pos[order] = ranks // C
    row = owner * SH + pos

    # edges + self-loops
    es = np.concatenate([src, np.arange(n_nodes)])
    ed = np.concatenate([dst, np.arange(n_nodes)])
    eo = owner[ed]                  # owning core (by dst)
    ep = pos[ed]
    er = row[es]                    # table row of source
    et = ep // P                    # dst tile
    el = ep % P                     # dst lane
    ew = er // WIN                  # source window
    ei = (er - ew * WIN).astype(np.int64)  # int16 gather index

    # counts per (core, window, tile)
    key = (eo * NW + ew) * T + et
    cnt = np.bincount(key, minlength=C * NW * T).reshape(C, NW, T)
    cmax = cnt.max(axis=0)                      # [NW, T]
    rlen = -(-cmax // P) * P                    # padded run length, [NW, T]
    run_start = np.zeros((NW, T), np.int64)     # slot offset of each run
    flat = rlen.reshape(-1)
    starts = np.concatenate([[0], np.cumsum(flat)[:-1]])
    run_start = starts.reshape(NW, T)
    slots_tot = int(flat.sum())
    NCH = slots_tot // P

    # instructions: chop each window segment into <=K slot pieces
    wseg = rlen.sum(axis=1)                     # slots per window
    wstart = np.concatenate([[0], np.cumsum(wseg)[:-1]])
    instrs = []                                 # (w, slot0, nslots)
    for w in range(NW):
        s0, s1 = int(wstart[w]), int(wstart[w] + wseg[w])
        while s0 < s1:
            n = min(K, s1 - s0)
            instrs.append((w, s0, n))
            s0 += n
    # chunk -> instr index
    instr_slot0 = np.array([s for (_, s, _) in instrs])
    NI = len(instrs)

    # runs in stream order: (w, t, chunk0, nchunks)
    runs = []
    first_for_tile = {}
    for w in range(NW):
        for t in range(T):
            if rlen[w, t] == 0:
                continue
            ch0 = int(run_start[w, t]) // P
            nch = int(rlen[w, t]) // P
            first = t not in first_for_tile
            first_for_tile[t] = True
            runs.append((w, t, ch0, nch, first))

    # per-core idx + lane streams in the shared layout
    idx_flat = np.zeros((C, slots_tot), np.int16)
    lane_flat = np.full((C, slots_tot), 255, np.int16)
    eorder = np.lexsort((ei, et, ew, eo))
    so, sw, st_, sl, si = (eo[eorder], ew[eorder], et[eorder],
                           el[eorder], ei[eorder])
    grp = (so * NW + sw) * T + st_
    newgrp = np.ones(len(grp), dtype=bool)
    newgrp[1:] = grp[1:] != grp[:-1]
    gstart = np.where(newgrp)[0]
    within = np.arange(len(grp)) - np.repeat(
        gstart, np.diff(np.concatenate([gstart, [len(grp)]])))
    slot = run_start[sw, st_] + within
    idx_flat[so, slot] = si.astype(np.int16)
    lane_flat[so, slot] = sl.astype(np.int16)

    # wrapped int16 idx: per instruction, k -> [k%16, off + k//16]
    NW16 = slots_tot // 16
    idx_wrap = np.zeros((C, 16, NW16), np.int16)
    for (w, s0, n) in instrs:
        blk = idx_flat[:, s0:s0 + n].reshape(C, n // 16, 16)
        idx_wrap[:, :, s0 // 16:(s0 + n) // 16] = blk.transpose(0, 2, 1)
    idx_wrap = np.tile(idx_wrap, (1, 8, 1))     # replicate to 128 partitions

    # lanes in [128, NCH] chunk-column layout (bf16 on device)
    lanes = lane_flat.reshape(C, NCH, P).transpose(0, 2, 1)  # [C,128,NCH]

    # dinv in shard order per core
    nodes_by_cp = np.full((C, per), -1, dtype=np.int64)
    nodes_by_cp[owner, pos] = np.arange(n_nodes)
    dinv_s = np.zeros((C, P, T_all), np.float32)
    for c in range(C):
        fulls = np.zeros(SH, np.float32)
        fulls[:per] = dinv[nodes_by_cp[c]]
        dinv_s[c] = fulls.reshape(T_all, P).T

    meta = dict(per=per, SH=SH, T=T, T_all=T_all, TAB=TAB, NW=NW, K=K,
                NI=NI, NCH=NCH, NW16=NW16, slots_tot=slots_tot,
                instrs=instrs, runs=runs, nodes_by_cp=nodes_by_cp,
                dinv=dinv, wseg=wseg, wstart=wstart)
    data = dict(idx_wrap=idx_wrap, lanes=lanes, dinv_s=dinv_s,
                lane_flat=lane_flat, idx_flat=idx_flat)
    return meta, data


def _build_nc2(meta, phases=("p1", "ag1", "l1", "ag2", "l2"), reps=1,
               scratch=65536, tiny_out=False):
    phases = set(phases)
    SH, T, T_all, TAB = meta["SH"], meta["T"], meta["T_all"], meta["TAB"]
    NW, K, NI, NCH, NW16 = (meta["NW"], meta["K"], meta["NI"], meta["NCH"],
                            meta["NW16"])
    instrs, runs = meta["instrs"], meta["runs"]
    J = K // P

    nc = bacc.Bacc("TRN2", target_bir_lowering=False, debug=False,
                   num_devices=C, dynamic_dma_scratch_size=scratch,
                   num_swdge_queues=4)
    xT = nc.dram_tensor("xT", [D, SH], BF16, kind="ExternalInput").ap()
    w1 = nc.dram_tensor("w1", [D, H], BF16, kind="ExternalInput").ap()
    w2p = nc.dram_tensor("w2p", [H, H], F32, kind="ExternalInput").ap()
    b1r = nc.dram_tensor("b1r", [P, H], F32, kind="ExternalInput").ap()
    b2r = nc.dram_tensor("b2r", [P, H], F32, kind="ExternalInput").ap()
    c2r = nc.dram_tensor("c2r", [P, H], F32, kind="ExternalInput").ap()
    ident = nc.dram_tensor("ident", [P, P], F32, kind="ExternalInput").ap()
    iota = nc.dram_tensor("iota", [P, P], BF16, kind="ExternalInput").ap()
    dinvs = nc.dram_tensor("dinvs", [P, T_all], F32, kind="ExternalInput").ap()
    lanesd = nc.dram_tensor("lanes", [P, NCH], BF16, kind="ExternalInput").ap()
    idxd = nc.dram_tensor("idx", [P, NW16], I16, kind="ExternalInput").ap()
    out = nc.dram_tensor("out", [P, H] if tiny_out else [SH, H], F32,
                         kind="ExternalOutput").ap()

    with tile.TileContext(nc) as tc:
        with (
            tc.tile_pool(name="cst", bufs=1) as cst,
            tc.tile_pool(name="sb", bufs=3) as sb,
            tc.tile_pool(name="gp", bufs=4) as gp,
            tc.tile_pool(name="sp", bufs=4) as sp,
            tc.tile_pool(name="ps", bufs=2, space="PSUM") as ps,
            tc.tile_pool(name="pt", bufs=2, space="PSUM") as pt_pool,
            tc.tile_pool(name="dram", bufs=1, space="DRAM") as dram,
        ):
            tab1l = dram.tile([SH, P], BF16)
            h2l = dram.tile([SH, P], BF16)
            tab1c = dram.tile([TAB, P], BF16, name="tab1c")
            tab2c = dram.tile([TAB, P], BF16, name="tab2c")
            outd = dram.tile([SH, H], F32, name="outd") if tiny_out else out

            # constants
            w1t = cst.tile([P, 4 * H], BF16)
            for f in range(4):
                nc.sync.dma_start(w1t[:, f * H:(f + 1) * H],
                                  w1[f * P:(f + 1) * P, :])
            w2pt = cst.tile([H, H], F32)
            nc.sync.dma_start(w2pt[:], w2p[:, :])
            b1t = cst.tile([P, H], F32)
            nc.sync.dma_start(b1t[:], b1r[:, :])
            b2t = cst.tile([P, H], F32)
            nc.sync.dma_start(b2t[:], b2r[:, :])
            c2t = cst.tile([P, H], F32)
            nc.sync.dma_start(c2t[:], c2r[:, :])
            idt = cst.tile([P, P], F32)
            nc.sync.dma_start(idt[:], ident[:, :])
            iot = cst.tile([P, P], BF16)
            nc.sync.dma_start(iot[:], iota[:, :])
            dst_ = cst.tile([P, T_all], F32)
            nc.sync.dma_start(dst_[:], dinvs[:, :])
            lant = cst.tile([P, NCH], BF16)
            nc.sync.dma_start(lant[:], lanesd[:, :])
            ixt = cst.tile([P, NW16], I16)
            nc.sync.dma_start(ixt[:], idxd[:, :])
            acc = cst.tile([P, T * H], F32)

            env = dict(locals())
            for _rep in range(reps):
                tag = f"r{_rep}" if _rep else ""
                env["tab1"] = dram.tile([TAB, P], BF16, addr_space="Shared",
                                        name=f"tab1{tag}", tag=f"tab1{tag}")
                env["tab2"] = dram.tile([TAB, P], BF16, addr_space="Shared",
                                        name=f"tab2{tag}", tag=f"tab2{tag}")
                _body2(nc, tc, phases, meta, env)

    nc.compile()
    return nc


def _gather_layer(nc, meta, env, tab, which, lvl=2, shared_tab=None):
    """Issue gathers + S-gen + PE accumulation into acc for one layer."""
    instrs, runs = meta["instrs"], meta["runs"]
    K, NW = meta["K"], meta["NW"]
    TAB = meta["TAB"]
    J = K // P
    gp, sp, ps = env["gp"], env["sp"], env["ps"]
    ixt, lant, iot, acc = env["ixt"], env["lant"], env["iot"], env["acc"]

    instr_slot0 = [s for (_, s, _) in instrs]
    gtiles = {}
    stiles = {}

    def ensure_instr(i):
        if i in gtiles:
            return
        (w, s0, n) = instrs[i]
        jn = n // P
        wsz = min(WIN, TAB - w * WIN)
        srctab = shared_tab if (shared_tab is not None and w == 0) else tab
        g = gp.tile([P, J * P], BF16, tag="g", name="g")
        nc.gpsimd.dma_gather(
            g[:, :jn * P].rearrange("p (j f) -> p j f", f=P),
            srctab[w * WIN: w * WIN + wsz, :],
            ixt[:, s0 // 16:(s0 + n) // 16],
            n, n, P, queue_num=i % 4)
        s = sp.tile([P, J * P], BF16, tag="s", name="s")
        ch0 = s0 // P
        if lvl >= 1:
            nc.vector.tensor_tensor(
                out=s[:, :jn * P].rearrange("p (j f) -> p j f", f=P),
                in0=lant[:, ch0:ch0 + jn].unsqueeze(2).broadcast_to((P, jn, P)),
                in1=iot[:].unsqueeze(1).broadcast_to((P, jn, P)),
                op=mybir.AluOpType.is_equal)
        gtiles[i] = g
        stiles[i] = s

    import bisect
    if lvl < 2:
        for i in range(len(instrs)):
            ensure_instr(i)
        return
    for (w, t, ch0, nch, first) in runs:
        pp = ps.tile([P, H], F32, tag="agg", name="pp")
        for k in range(nch):
            ch = ch0 + k
            i = bisect.bisect_right(instr_slot0, ch * P) - 1
            ensure_instr(i)
            lc = (ch * P - instr_slot0[i]) // P
            nc.tensor.matmul(
                pp[:],
                lhsT=stiles[i][:, lc * P:(lc + 1) * P],
                rhs=gtiles[i][:, lc * P:lc * P + H],
                start=(k == 0), stop=(k == nch - 1))
        if first:
            nc.scalar.activation(acc[:, t * H:(t + 1) * H], pp[:],
                                 mybir.ActivationFunctionType.Copy)
        else:
            nc.vector.tensor_add(acc[:, t * H:(t + 1) * H],
                                 acc[:, t * H:(t + 1) * H], pp[:])


def _body2(nc, tc, phases, meta, env):
    SH, T, T_all, TAB = meta["SH"], meta["T"], meta["T_all"], meta["TAB"]
    sb, ps, pt_pool = env["sb"], env["ps"], env["pt_pool"]
    xT, out = env["xT"], env["out"]
    outd = env["outd"]
    tab1c, tab2c = env["tab1c"], env["tab2c"]
    tab1l, h2l = env["tab1l"], env["h2l"]
    tab1, tab2 = env["tab1"], env["tab2"]
    w1t, w2pt = env["w1t"], env["w2pt"]
    b1t, b2t, c2t = env["b1t"], env["b2t"], env["c2t"]
    idt, dst_, acc = env["idt"], env["dst_"], env["acc"]

    NST = SH // 512
    # ---- P1: this core's shard of x @ W1, scaled by dinv ----
    if "p1" in phases:
        for st in range(NST):
            xt = sb.tile([P, 4 * D], BF16, tag="xt")
            for f in range(4):
                nc.sync.dma_start(
                    xt[:, f * D:(f + 1) * D],
                    xT[f * P:(f + 1) * P, st * 512:(st + 1) * 512])
            for g4 in range(4):
                pp = ps.tile([P, H], F32, tag="p1ps")
                for f in range(4):
                    nc.tensor.matmul(
                        pp[:],
                        lhsT=xt[:, f * D + g4 * P: f * D + (g4 + 1) * P],
                        rhs=w1t[:, f * H:(f + 1) * H],
                        start=(f == 0), stop=(f == 3))
                g = st * 4 + g4
                ht = sb.tile([P, P], BF16, tag="ht")
                nc.scalar.activation(ht[:, 0:H], pp[:],
                                     mybir.ActivationFunctionType.Copy,
                                     scale=dst_[:, g:g + 1])
                nc.sync.dma_start(tab1l[g * P:(g + 1) * P, :], ht[:])

    if "ag1" in phases:
        nc.gpsimd.collective_compute(
            "AllGather", mybir.AluOpType.bypass,
            replica_groups=[list(range(C))],
            ins=[tab1l.opt()], outs=[tab1.opt()])

    # ---- Layer 1: gather-accumulate + epilogue ----
    t1src = tab1 if NOCOPY else tab1c
    # window 0 gathers read Shared directly; windows 1-3 are copied to
    # Local (faster gathers) on three different HWDGE queues, overlapping
    # the window-0 gathers.
    engs = [nc.sync, nc.scalar, nc.sync]
    if not NOCOPY and phases & {"l1", "l1g", "l1ge"}:
        for w in range(1, 4):
            base = w * WIN
            wsz = min(WIN, TAB - base)
            for h in range(2):
                if h * (wsz // 2) >= wsz:
                    continue
                lo = base + h * (wsz // 2)
                hi = base + wsz if h == 1 else lo + wsz // 2
                engs[w - 1].dma_start(tab1c[lo:hi, :], tab1[lo:hi, :])
    st1 = tab1 if not NOCOPY else None
    if "l1g" in phases:
        _gather_layer(nc, meta, env, t1src, "a", lvl=0, shared_tab=st1)
    if "l1ge" in phases:
        _gather_layer(nc, meta, env, t1src, "a", lvl=1, shared_tab=st1)
    if "l1" in phases:
        _gather_layer(nc, meta, env, t1src, "a", shared_tab=st1)
        for t in range(T):
            red = sb.tile([P, H], F32, tag="red")
            nc.vector.tensor_scalar_mul(red[:], acc[:, t * H:(t + 1) * H],
                                        dst_[:, t:t + 1])
            nc.vector.tensor_add(red[:], red[:], b1t[:])
            nc.vector.tensor_scalar_max(red[:], red[:], 0.0)
            pt = pt_pool.tile([H, P], F32, tag="pst")
            nc.tensor.transpose(pt[:], red[:], idt[:])
            rt = sb.tile([H, P], F32, tag="rt")
            nc.scalar.activation(rt[:], pt[:],
                                 mybir.ActivationFunctionType.Copy)
            p2 = pt_pool.tile([P, H], F32, tag="ps2")
            nc.tensor.matmul(p2[:], lhsT=rt[:], rhs=w2pt[:],
                             start=True, stop=True)
            h2f = sb.tile([P, H], F32, tag="h2f")
            nc.vector.tensor_add(h2f[:], p2[:], c2t[:])
            h2t = sb.tile([P, P], BF16, tag="h2t")
            nc.vector.tensor_scalar_mul(h2t[:, 0:H], h2f[:],
                                        dst_[:, t:t + 1])
            nc.sync.dma_start(h2l[t * P:(t + 1) * P, :], h2t[:])

    if "ag2" in phases:
        nc.gpsimd.collective_compute(
            "AllGather", mybir.AluOpType.bypass,
            replica_groups=[list(range(C))],
            ins=[h2l.opt()], outs=[tab2.opt()])

    # ---- liveness touch: copy a slice of the deepest phase's output ----
    if "l2" not in phases:
        deep = None
        if "ag2" in phases:
            deep = tab2
        elif "l1" in phases:
            deep = h2l
        elif "ag1" in phases:
            deep = tab1
        elif "p1" in phases:
            deep = tab1l
        if deep is not None:
            tch = sb.tile([P, H], BF16, tag="tch", name="tch")
            nc.sync.dma_start(tch[:], deep[0:P, 0:H])
            tchf = sb.tile([P, H], F32, tag="tchf", name="tchf")
            nc.vector.tensor_copy(tchf[:], tch[:])
            nc.sync.dma_start(out[0:P, :], tchf[:])

    # ---- Layer 2 ----
    if "l2" in phases:
        if not NOCOPY:
            for w in range(1, 4):
                base = w * WIN
                wsz = min(WIN, TAB - base)
                for h in range(2):
                    if h * (wsz // 2) >= wsz:
                        continue
                    lo = base + h * (wsz // 2)
                    hi = base + wsz if h == 1 else lo + wsz // 2
                    engs[w - 1].dma_start(tab2c[lo:hi, :], tab2[lo:hi, :])
        _gather_layer(nc, meta, env, tab2 if NOCOPY else tab2c, "b",
                      shared_tab=tab2 if not NOCOPY else None)
        for t in range(T):
            red = sb.tile([P, H], F32, tag="red2")
            nc.vector.tensor_scalar_mul(red[:], acc[:, t * H:(t + 1) * H],
                                        dst_[:, t:t + 1])
            nc.vector.tensor_add(red[:], red[:], b2t[:])
            ot = sb.tile([P, H], F32, tag="ot")
            nc.vector.tensor_scalar_max(ot[:], red[:], 0.0)
            nc.sync.dma_start(outd[t * P:(t + 1) * P, :], ot[:])
        if outd is not out:
            tc2 = sb.tile([P, H], F32, tag="tc2", name="tc2")
            nc.sync.dma_start(tc2[:], outd[0:P, :])
            nc.sync.dma_start(out[0:P, :], tc2[:])


def _impl2(x, edge_index, W1, b1, W2, b2, gamma, beta, run_mean, run_var,
           n_nodes):
    x = np.asarray(x, np.float32)
    W1 = np.asarray(W1, np.float32)
    b1 = np.asarray(b1, np.float32)
    W2 = np.asarray(W2, np.float32)
    b2 = np.asarray(b2, np.float32)
    gamma = np.asarray(gamma, np.float32)
    beta = np.asarray(beta, np.float32)
    run_mean = np.asarray(run_mean, np.float32)
    run_var = np.asarray(run_var, np.float32)

    meta, data = _plan2(n_nodes, np.asarray(edge_index))
    per, SH, T_all = meta["per"], meta["SH"], meta["T_all"]

    s = gamma / np.sqrt(run_var + BN_EPS)
    t = beta - run_mean * s
    W2p = (W2 * s[:, None]).astype(np.float32)
    c2 = (t @ W2).astype(np.float32)

    b1rep = np.tile(b1[None, :], (P, 1)).astype(np.float32)
    b2rep = np.tile(b2[None, :], (P, 1)).astype(np.float32)
    c2rep = np.tile(c2[None, :], (P, 1)).astype(np.float32)
    identv = np.eye(P, dtype=np.float32)
    iotav = np.tile(np.arange(P, dtype=np.float32)[None, :],
                    (P, 1)).astype(BF)

    nodes_by_cp = meta["nodes_by_cp"]
    in_maps = []
    for c in range(C):
        xs = np.zeros((SH, D), np.float32)
        xs[:per] = x[nodes_by_cp[c]]
        in_maps.append({
            "xT": np.ascontiguousarray(xs.T).astype(BF),
            "w1": W1.astype(BF), "w2p": W2p,
            "b1r": b1rep, "b2r": b2rep, "c2r": c2rep,
            "ident": identv, "iota": iotav,
            "dinvs": np.ascontiguousarray(data["dinv_s"][c]),
            "lanes": np.ascontiguousarray(data["lanes"][c]).astype(BF),
            "idx": np.ascontiguousarray(data["idx_wrap"][c]),
        })

    nc = _build_nc2(meta)
    global _LAST_NC, _LAST_IN_MAPS, _LAST_META
    _LAST_NC, _LAST_IN_MAPS, _LAST_META = nc, in_maps, meta
    res = run_bass_kernel_spmd(nc, in_maps, core_ids=list(range(C))).results

    outf = np.zeros((n_nodes, H), np.float32)
    for c in range(C):
        outf[nodes_by_cp[c]] = res[c]["out"][:per]
    return outf


def kernel(x, edge_index, W1, b1, W2, b2, gamma, beta, run_mean, run_var):
    return _impl2(x, edge_index, W1, b1, W2, b2, gamma, beta, run_mean,
                  run_var, n_nodes=100000)
